# revision 1
# baseline (speedup 1.0000x reference)
"""GCN encoder (2x GCNConv + global max pool + 2x FC) on 8 TRN2 NeuronCores.

Strategy (sharding hint: node partitioning + halo exchange; graph-aligned
node sharding so pooling is core-local):
  - Nodes sharded contiguously at graph boundaries: rank r owns graphs
    [r*GR, (r+1)*GR) and a contiguous node range, padded to NMAX (mult 128).
  - Edges are routed to the rank owning their dst node. The edge pipeline is
    all-SBUF: per src-rank chunk, the (pre-scaled, bf16) feature table is
    DMA'd contiguously into SBUF; dma_gather with SBUF source produces
    feature-major messages; PE transposes them back to edge-major (fp32 in
    PSUM); dma_scatter_add with SBUF destination (parity-split CCE fp32 add)
    accumulates into SBUF slab accumulators. No random-access HBM traffic.
  - GCN normalization: norm = dinv[src]*dinv[dst]. Tables are pre-scaled by
    dinv (x host-side; h1s on device), so no per-edge scaling is needed; the
    dinv[dst] factor is applied per-node in the post phase:
      out_l = relu((dinv .* (agg + dinv .* h)) @ W_l + b_l)
  - Both layers share one index array: token = src - rank_base, wrapped 128
    across partitions (stripe s holds rows s*128..s*128+127), identically for
    the x table and the h1 table.
  - After layer 1, h1s = dinv .* h1 (bf16, wrap-128 layout) is AllGather'd
    (the halo is essentially everything for a random graph).
  - Max pooling: h2 is produced feature-major (h2T); per-graph max is a fixed
    number of clamped fixed-width windowed reduce_max ops (overlap idempotent
    for max) whose start columns are per-core data loaded into registers
    (SPMD-uniform program). Tiny AllGather of pooled partials, then the
    (replicated) FC head runs on every core.

  HW-calibrated constraints baked in: dma_scatter_add requires unique dst
  indices per instruction (CCE read-modify-write races on duplicates); the
  SWDGE descriptor ring (dynamic_dma_scratch_size/16 entries) must hold ~2
  descriptors per valid index for the in-flight gather+scatter window, hence
  the per-tile valid cap of 960 with a 4096-entry ring.
"""

import math
from contextlib import ExitStack

import numpy as np

import concourse.bass as bass
import concourse.bacc as bacc
import concourse.mybir as mybir
import concourse.tile as tile
from concourse import library_config

F32 = mybir.dt.float32
BF16 = mybir.dt.bfloat16
I16 = mybir.dt.int16

R = 8          # NeuronCores
C = 8          # src chunks (one per owner rank)
F = 128        # in dim == hidden
H2 = 256       # 2*hidden
FCD = 512      # fc1 out
PROJ = 128     # fc2 out


# ----------------------------------------------------------------- host prep

def _wrap16(a: np.ndarray) -> np.ndarray:
    """[T] int -> [128, T//16] int16: idx j at (j%16, j//16), replicated to
    all 8 groups of 16 partitions."""
    T = a.shape[0]
    w = np.ascontiguousarray(a.reshape(T // 16, 16).T).astype(np.int16)
    return np.tile(w, (8, 1))


def preprocess(x, edge_index, batch, te=2048, win=None):
    x = np.asarray(x, dtype=np.float32)
    src = np.asarray(edge_index[0], dtype=np.int64)
    dst = np.asarray(edge_index[1], dtype=np.int64)
    batch = np.asarray(batch, dtype=np.int64)
    N = x.shape[0]
    G = int(batch.max()) + 1 if batch.size else 1
    G = max(G, R)  # at least one graph per rank
    assert G % R == 0, f"graphs {G} not divisible by {R}"
    GR = G // R

    sizes = np.bincount(batch, minlength=G)
    gstart = np.concatenate([[0], np.cumsum(sizes)])  # [G+1]
    rb = gstart[::GR].copy()                          # [R+1] rank node bounds
    assert rb[-1] == N
    Nc = np.diff(rb)
    NMAX = int(math.ceil(max(int(Nc.max()), 128) / 128) * 128)
    assert NMAX <= 32766, "int16 gather index overflow"
    NTL = NMAX // 128
    # +dump slab(s) for fake edges; round slab count to even for the
    # parity-split SBUF accumulators
    AGG_ROWS = NMAX + 128 if (NTL + 1) % 2 == 0 else NMAX + 256

    deg = np.bincount(dst, minlength=N).astype(np.float64) + 1.0
    dinv = (1.0 / np.sqrt(deg)).astype(np.float32)

    # per-edge routing: chunk = rank owning the src row
    rk_dst = np.searchsorted(rb, dst, side="right") - 1
    ch_src = np.searchsorted(rb, src, side="right") - 1
    srel_all = (src - rb[ch_src]).astype(np.int64)    # token in chunk table
    dstrel_all = (dst - rb[rk_dst]).astype(np.int64)

    # --- edge -> tile assignment: dma_scatter_add races on duplicate dst
    # indices within one instruction, so each tile must have UNIQUE dsts.
    cnt = np.zeros((R, C), dtype=np.int64)
    occ_max = np.zeros(C, dtype=np.int64)
    for r in range(R):
        for c in range(C):
            m = (rk_dst == r) & (ch_src == c)
            cnt[r, c] = int(m.sum())
            if cnt[r, c]:
                occ_max[c] = max(occ_max[c],
                                 int(np.bincount(dstrel_all[m]).max()))

    cap = min(te // 2 - 128, 960)  # SWDGE ring: ~2 descs per valid idx
    NT_c = []
    for c in range(C):
        t_need = max(int(math.ceil(cnt[:, c].max() * 1.08 / cap)),
                     int(occ_max[c]), 1)
        while True:  # grow until every (rank, tile) fits with the fake edge
            ok = True
            for r in range(R):
                m = (rk_dst == r) & (ch_src == c)
                ddm = dstrel_all[m]
                if len(ddm) == 0:
                    continue
                order = np.argsort(ddm, kind="stable")
                sorted_d = ddm[order]
                runstart = np.r_[0, np.flatnonzero(np.diff(sorted_d)) + 1]
                occ_sorted = np.arange(len(ddm)) - np.repeat(
                    runstart, np.diff(np.r_[runstart, len(ddm)]))
                occ = np.zeros(len(ddm), dtype=np.int64)
                occ[order] = occ_sorted
                tf = (occ + (ddm * 2654435761 % t_need)) % t_need
                if int(np.bincount(tf, minlength=t_need).max()) + 1 > cap:
                    ok = False
                    break
            if ok:
                break
            t_need += 1
        NT_c.append(t_need)
    NT = sum(NT_c)
    chunk_of = sum(([c] * NT_c[c] for c in range(C)), [])

    es = np.full((R, NT, 128, te // 16), -1, dtype=np.int16)
    ed = np.full((R, NT, 128, te // 16), -1, dtype=np.int16)
    ecnt = np.zeros((R, NT), dtype=np.int32)

    for r in range(R):
        t0 = 0
        for c in range(C):
            m = (rk_dst == r) & (ch_src == c)
            T_c = NT_c[c]
            sm = srel_all[m]
            ddm = dstrel_all[m]
            # occurrence index of each edge within its dst group
            order = np.argsort(ddm, kind="stable")
            occ = np.zeros(len(ddm), dtype=np.int64)
            if len(ddm):
                sorted_d = ddm[order]
                runstart = np.r_[0, np.flatnonzero(np.diff(sorted_d)) + 1]
                occ_sorted = np.arange(len(ddm)) - np.repeat(
                    runstart, np.diff(np.r_[runstart, len(ddm)]))
                occ[order] = occ_sorted
            tile_of = (occ + (ddm * 2654435761 % T_c)) % T_c
            for t in range(T_c):
                sel = tile_of == t
                k = int(sel.sum())
                assert k + 1 <= te // 2 - 64, (
                    f"tile overflow r{r} c{c} t{t}: {k + 1}")
                sv = np.full(te, -1, dtype=np.int64)
                dd = np.full(te, -1, dtype=np.int64)
                # ascending src order within the tile: the gather's HBM
                # reads walk forward through the chunk (row-buffer locality)
                o2 = np.argsort(sm[sel], kind="stable")
                sv[:k] = sm[sel][o2]
                dd[:k] = ddm[sel][o2]
                # one fake edge so every tile has >=1 valid index; it adds
                # table row 0 into the (never-read) dump slab
                sv[k] = 0
                dd[k] = NMAX + ((t0 + t) % 128)
                es[r, t0 + t] = _wrap16(sv)
                ed[r, t0 + t] = _wrap16(dd)
                ecnt[r, t0 + t] = k + 1
            t0 += T_c

    # pre-scaled gather table: xg = dinv .* x (the dinv[dst] factor is
    # applied per-node in the post phase)
    xg = (dinv[:, None] * x).astype(np.float32)

    # per-rank padded node slices (fp32, pre-scaled) for the post phase
    xc = np.zeros((R, NMAX, F), dtype=np.float32)
    dvc = np.zeros((R, NMAX, 1), dtype=np.float32)
    for r in range(R):
        n = int(Nc[r])
        xc[r, :n] = xg[rb[r]:rb[r + 1]]
        dvc[r, :n, 0] = dinv[rb[r]:rb[r + 1]]

    # pooling windows: every graph gets exactly WPG windows of width WIN with
    # data-driven start columns (clamped overlapping windows; idempotent for
    # max). Uniform structure across cores; starts differ per core.
    min_sz = int(sizes.min())
    assert min_sz > 0, "empty graph not supported"
    if win is None:
        win = 512
    win = int(min(win, min_sz))
    wpg = int(math.ceil(int(sizes.max()) / win))
    wstart = np.zeros((R, GR * wpg), dtype=np.int32)
    for r in range(R):
        for j in range(GR):
            g = r * GR + j
            a0 = int(gstart[g] - rb[r])
            sz = int(sizes[g])
            for k in range(wpg):
                wstart[r, j * wpg + k] = min(a0 + k * win, a0 + sz - win)

    cfg = dict(
        N=N, G=G, GR=GR, NMAX=NMAX, NTL=NTL, AGG_ROWS=AGG_ROWS, TE=te,
        NT=NT, chunk_of=chunk_of, WIN=win, WPG=wpg,
        l1_base=[int(rb[c]) for c in range(C)],
        l1_size=[int(Nc[c]) for c in range(C)],
    )
    per_core = dict(es=es, ed=ed, ecnt=ecnt[:, None, :],
                    xc=xc, dvc=dvc, xg=xg,
                    wstart=wstart[:, :, None].astype(np.int32))
    return cfg, per_core


def make_in_maps(cfg, per_core, x, W1, b1, W2, b2, fc1_w, fc1_b, fc2_w, fc2_b):
    w1 = np.asarray(W1, dtype=np.float32)
    b1v = np.asarray(b1, dtype=np.float32).reshape(1, F)
    w2 = np.asarray(W2, dtype=np.float32)
    b2s = np.asarray(b2, dtype=np.float32).reshape(2, 128).T.copy()      # [128,2]
    f1w = np.asarray(fc1_w, dtype=np.float32).reshape(2, 128, FCD)
    f1w = np.ascontiguousarray(f1w.transpose(1, 0, 2)).reshape(128, 2 * FCD)
    f1b = np.asarray(fc1_b, dtype=np.float32).reshape(4, 128).T.copy()   # [128,4]
    f2w = np.asarray(fc2_w, dtype=np.float32).reshape(4, 128, PROJ)
    f2w = np.ascontiguousarray(f2w.transpose(1, 0, 2)).reshape(128, 4 * PROJ)
    f2b = np.asarray(fc2_b, dtype=np.float32).reshape(1, PROJ)
    ident = np.eye(128, dtype=np.float32)

    shared = dict(x=per_core["xg"], w1=w1, b1=b1v, w2=w2, b2s=b2s,
                  f1w=f1w, f1b=f1b, f2w=f2w, f2b=f2b, ident=ident)
    in_maps = []
    for r in range(R):
        m = dict(shared)
        m["xc"] = per_core["xc"][r]
        m["dvc"] = per_core["dvc"][r]
        m["es"] = per_core["es"][r]
        m["ed"] = per_core["ed"][r]
        m["ecnt"] = per_core["ecnt"][r]
        m["wstart"] = per_core["wstart"][r]
        in_maps.append(m)
    return in_maps


# ------------------------------------------------------------------- builder

def build_program(cfg, debug_outs=False, variant=""):
    N = cfg["N"]; G = cfg["G"]; GR = cfg["GR"]
    NMAX = cfg["NMAX"]; NTL = cfg["NTL"]; AGG_ROWS = cfg["AGG_ROWS"]
    TE = cfg["TE"]; NT = cfg["NT"]; chunk_of = cfg["chunk_of"]
    WIN = cfg["WIN"]; WPG = cfg["WPG"]
    TE16 = TE // 16; TE128 = TE // 128
    RG = [list(range(R))]

    nc = bacc.Bacc("TRN2", target_bir_lowering=False, debug=False,
                   num_devices=R, dynamic_dma_scratch_size=65536)

    x_d = nc.dram_tensor("x", [N, F], F32, kind="ExternalInput")
    xc_d = nc.dram_tensor("xc", [NMAX, F], F32, kind="ExternalInput")
    dvc_d = nc.dram_tensor("dvc", [NMAX, 1], F32, kind="ExternalInput")
    es_d = nc.dram_tensor("es", [NT, 128, TE16], I16, kind="ExternalInput")
    ecnt_d = nc.dram_tensor("ecnt", [1, NT], mybir.dt.int32,
                            kind="ExternalInput")
    ed_d = nc.dram_tensor("ed", [NT, 128, TE16], I16, kind="ExternalInput")
    w1_d = nc.dram_tensor("w1", [F, F], F32, kind="ExternalInput")
    b1_d = nc.dram_tensor("b1", [1, F], F32, kind="ExternalInput")
    w2_d = nc.dram_tensor("w2", [F, H2], F32, kind="ExternalInput")
    b2s_d = nc.dram_tensor("b2s", [128, 2], F32, kind="ExternalInput")
    f1w_d = nc.dram_tensor("f1w", [128, 2 * FCD], F32, kind="ExternalInput")
    f1b_d = nc.dram_tensor("f1b", [128, 4], F32, kind="ExternalInput")
    f2w_d = nc.dram_tensor("f2w", [128, 4 * PROJ], F32, kind="ExternalInput")
    f2b_d = nc.dram_tensor("f2b", [1, PROJ], F32, kind="ExternalInput")
    ident_d = nc.dram_tensor("ident", [128, 128], F32, kind="ExternalInput")
    wstart_d = nc.dram_tensor("wstart", [GR * WPG, 1], mybir.dt.int32,
                              kind="ExternalInput")

    h1s_d = nc.dram_tensor("h1s", [NMAX, F], F32)
    h1full_d = nc.dram_tensor("h1full", [R * NMAX, F], F32,
                              addr_space="Shared")
    h2t_d = nc.dram_tensor("h2t", [2, 128, NMAX], F32)
    gmax_d = nc.dram_tensor("gmax", [2, 128, GR], F32)
    gpool_d = nc.dram_tensor("gpool", [R, 2, 128, GR], F32, addr_space="Shared")
    out_d = nc.dram_tensor("out", [G, PROJ], F32, kind="ExternalOutput")
    h1fX_d = (nc.dram_tensor("h1fX", [R * NMAX, F], F32,
                             addr_space="Shared")
              if variant.startswith("agx") else None)
    dbg = {}
    if debug_outs:
        for nm, shp, dt_ in [("dbg_h1s", [NMAX, F], F32),
                             ("dbg_agg1", [NMAX, F], F32),
                             ("dbg_h2t", [2, 128, NMAX], F32),
                             ("dbg_gmax", [2, 128, GR], F32),
                             ("dbg_gpool", [R, 2, 128, GR], F32)]:
            dbg[nm] = nc.dram_tensor(nm, shp, dt_, kind="ExternalOutput")

    with tile.TileContext(nc, num_cores=R) as tc, ExitStack() as stk:
        cp = stk.enter_context(tc.tile_pool(name="consts", bufs=1))
        w1s = cp.tile([F, F], F32)
        b1s = cp.tile([1, F], F32)
        w2s = cp.tile([F, H2], F32)
        b2ss = cp.tile([128, 2], F32)
        f1ws = cp.tile([128, 2 * FCD], F32)
        f1bs = cp.tile([128, 4], F32)
        f2ws = cp.tile([128, 4 * PROJ], F32)
        f2bs = cp.tile([1, PROJ], F32)
        idents = cp.tile([128, 128], F32)
        ones = cp.tile([1, 128], F32)
        # SBUF edge accumulators: NSET independent sets (tiles alternate
        # sets to break scatter->scatter WAW serialization), each split into
        # even/odd slab halves (dma_scatter_add parity-split layout: slab =
        # idx//128 goes to half slab%2 at free offset (slab//2)*F)
        SLABS = AGG_ROWS // 128
        NSET = 2
        agg_t = [[cp.tile([128, (SLABS // 2) * F], F32, tag=f"agg{s}_{p}",
                          name=f"agg{s}_{p}") for p in range(2)]
                 for s in range(NSET)]

        nc.gpsimd.load_library(library_config.mlp)
        nc.sync.dma_start(w1s[:], w1_d[:])
        nc.sync.dma_start(b1s[:], b1_d[:])
        nc.sync.dma_start(w2s[:], w2_d[:])
        nc.sync.dma_start(b2ss[:], b2s_d[:])
        nc.sync.dma_start(f1ws[:], f1w_d[:])
        nc.sync.dma_start(f1bs[:], f1b_d[:])
        nc.sync.dma_start(f2ws[:], f2w_d[:])
        nc.sync.dma_start(f2bs[:], f2b_d[:])
        nc.sync.dma_start(idents[:], ident_d[:])
        nc.vector.memset(ones[:], 1.0)

        def agg_clear():
            for s in range(NSET):
                for p in range(2):
                    nc.vector.memset(agg_t[s][p][:], 0.0)

        def agg_slab(s, i):
            """AP of agg set s, 128-row slab i."""
            return agg_t[s][i % 2][:, (i // 2) * F:(i // 2 + 1) * F]

        agg_clear()

        # ---- edge pass helper
        ip = stk.enter_context(tc.tile_pool(name="idx", bufs=3))
        mp = stk.enter_context(tc.tile_pool(name="msg", bufs=2))
        ecs = cp.tile([1, NT], mybir.dt.int32)
        nc.sync.dma_start(ecs[:], ecnt_d[:])

        def edge_pass(table_d, bases, sizes_, lbl=""):
            for t in range(NT):
                c = chunk_of[t]
                sidx = ip.tile([128, TE16], I16, tag="esrc")
                nc.sync.dma_start(sidx[:], es_d[t])
                didx = ip.tile([128, TE16], I16, tag="edst")
                nc.sync.dma_start(didx[:], ed_d[t])
                msg = mp.tile([128, TE128, 128], F32, tag="msg")
                if lbl == "" and t < 2:  # init the pool slots once: gather
                    nc.vector.memset(msg[:], 0.0)  # leaves -1-idx slots stale
                tab = table_d[bases[c]:bases[c] + sizes_[c], :]
                s = t % NSET
                with nc.gpsimd.register(f"ec{lbl}{t}") as rg:
                    nc.gpsimd.reg_load(rg, ecs[0:1, t:t + 1])
                    nv = nc.gpsimd.snap(rg)
                    nc.gpsimd.dma_gather(msg[:], tab, sidx[:], TE, nv, F)
                    nc.gpsimd.dma_scatter_add(
                        agg_t[s][0][:, :], msg[:], didx[:], TE, nv, F,
                        sbuf_tokens_per_rank=128, parity_reg=0,
                        out_ap_other=agg_t[s][1][:, :])

        # ---- layer 1 message passing
        l2_base = [c * NMAX for c in range(C)]
        l2_size = [NMAX] * C
        if variant.startswith("edge1x"):  # dups first; cleared before real
            for rp in range(int(variant[6:]) - 1):
                edge_pass(x_d, cfg["l1_base"], cfg["l1_size"], lbl=f"X{rp}")
            tc.strict_bb_all_engine_barrier()
            agg_clear()
        edge_pass(x_d, cfg["l1_base"], cfg["l1_size"])
        tc.strict_bb_all_engine_barrier()
        if debug_outs:
            for i in range(NTL):
                da = cp.tile([128, F], F32, tag="dbgag", name=f"dbgag{i}")
                nc.vector.tensor_add(da[:], agg_slab(0, i), agg_slab(1, i))
                nc.sync.dma_start(dbg["dbg_agg1"][i * 128:(i + 1) * 128, :],
                                  da[:])

        # ---- post 1: h1s = dinv*relu((dinv*(agg + dinv*x)) @ W1 + b1)
        pp = stk.enter_context(tc.tile_pool(name="post", bufs=3))
        tp = stk.enter_context(
            tc.tile_pool(name="tpsum", bufs=2, space="PSUM"))
        mmp = stk.enter_context(
            tc.tile_pool(name="mpsum", bufs=2, space="PSUM"))

        def post1(out_d_):
            for i in range(NTL):
                sl = slice(i * 128, (i + 1) * 128)
                ag = pp.tile([128, F], F32, tag="ag")
                nc.vector.tensor_add(ag[:], agg_slab(0, i), agg_slab(1, i))
                xt = pp.tile([128, F], F32, tag="xt")
                nc.sync.dma_start(xt[:], xc_d[sl, :])
                dv = pp.tile([128, 1], F32, tag="dv")
                nc.sync.dma_start(dv[:], dvc_d[sl, :])
                s = pp.tile([128, F], F32, tag="s")
                nc.vector.tensor_add(s[:], xt[:], ag[:])
                s2 = pp.tile([128, F], F32, tag="s2")
                nc.vector.tensor_scalar_mul(s2[:], s[:], dv[:])
                pt = tp.tile([128, 128], F32, tag="pt")
                nc.tensor.transpose(pt[:], s2[:], idents[:])
                aT = pp.tile([128, 128], F32, tag="aT")
                nc.vector.tensor_copy(aT[:], pt[:])
                p1 = mmp.tile([128, F], F32, tag="p1")
                nc.tensor.matmul(p1[:], aT[:], w1s[:], start=True, stop=False)
                nc.tensor.matmul(p1[:], ones[:], b1s[:], start=False,
                                 stop=True)
                h1 = pp.tile([128, F], F32, tag="h1")
                nc.scalar.activation(h1[:], p1[:],
                                     mybir.ActivationFunctionType.Relu)
                h1s = pp.tile([128, F], F32, tag="h1s")
                nc.vector.tensor_scalar_mul(h1s[:], h1[:], dv[:])
                nc.sync.dma_start(out_d_[sl, :], h1s[:])

        post1(h1s_d)
        if variant.startswith("postx"):
            for rp in range(int(variant[5:]) - 1):
                post1(h1s_d)

        tc.strict_bb_all_engine_barrier()
        if debug_outs:
            nc.sync.dma_start(dbg["dbg_h1s"][:, :], h1s_d[:, :])
        agg_clear()  # overlaps the collective (DVE vs DMA)
        nc.gpsimd.collective_compute(
            "AllGather", mybir.AluOpType.bypass, replica_groups=RG,
            ins=[h1s_d[:, :]], outs=[h1full_d[:, :]])
        if variant.startswith("agx"):
            for rp in range(int(variant[3:]) - 1):
                nc.gpsimd.collective_compute(
                    "AllGather", mybir.AluOpType.bypass, replica_groups=RG,
                    ins=[h1s_d[:, :]], outs=[h1fX_d[:, :]])
        tc.strict_bb_all_engine_barrier()

        # ---- layer 2 message passing (table pre-scaled by dinv)
        if variant.startswith("edge2x"):  # dups first; cleared before real
            for rp in range(int(variant[6:]) - 1):
                edge_pass(h1full_d, l2_base, l2_size, lbl=f"Y{rp}")
            tc.strict_bb_all_engine_barrier()
            agg_clear()
        edge_pass(h1full_d, l2_base, l2_size, lbl="b")
        tc.strict_bb_all_engine_barrier()

        # ---- post 2: h2T = relu(W2^T @ (dinv*(agg2 + h1s)) + b2), feature-major
        for i in range(NTL):
            sl = slice(i * 128, (i + 1) * 128)
            ag = pp.tile([128, F], F32, tag="ag")
            nc.vector.tensor_add(ag[:], agg_slab(0, i), agg_slab(1, i))
            hs = pp.tile([128, F], F32, tag="hsb")
            nc.sync.dma_start(hs[:], h1s_d[sl, :])
            dv = pp.tile([128, 1], F32, tag="dv")
            nc.sync.dma_start(dv[:], dvc_d[sl, :])
            s = pp.tile([128, F], F32, tag="s")
            nc.vector.tensor_add(s[:], hs[:], ag[:])
            s2 = pp.tile([128, F], F32, tag="s2")
            nc.vector.tensor_scalar_mul(s2[:], s[:], dv[:])
            pt = tp.tile([128, 128], F32, tag="pt")
            nc.tensor.transpose(pt[:], s2[:], idents[:])
            aT = pp.tile([128, 128], F32, tag="aT")
            nc.vector.tensor_copy(aT[:], pt[:])
            for h in range(2):
                p2 = mmp.tile([128, 128], F32, tag="p1")
                nc.tensor.matmul(p2[:], w2s[:, h * 128:(h + 1) * 128], aT[:],
                                 start=True, stop=True)
                h2t = pp.tile([128, 128], F32, tag="h1")
                nc.scalar.activation(h2t[:], p2[:],
                                     mybir.ActivationFunctionType.Relu,
                                     bias=b2ss[:, h:h + 1])
                nc.sync.dma_start(h2t_d[h, :, sl], h2t[:])

        tc.strict_bb_all_engine_barrier()

        if debug_outs:
            nc.sync.dma_start(dbg["dbg_h2t"][:, :, :], h2t_d[:, :, :])
        # ---- pooling: WPG fixed windows per graph, data-driven start columns
        gm = cp.tile([128, 2 * GR], F32)
        wp = stk.enter_context(tc.tile_pool(name="win", bufs=4))
        wsts = cp.tile([GR * WPG, 1], mybir.dt.int32)
        nc.sync.dma_start(wsts[:], wstart_d[:])
        gslots = cp.tile([128, 2 * GR * WPG], F32)
        pool_reps = (int(variant[5:]) if variant.startswith("poolx")
                     else 1)
        for rep in range(pool_reps):
            for j in range(GR):
                for k in range(WPG):
                    w = j * WPG + k
                    with nc.gpsimd.register(f"wst{rep}_{w}") as rg:
                        nc.gpsimd.reg_load(rg, wsts[w:w + 1, 0:1])
                        sv = nc.gpsimd.snap(rg)
                        for h in range(2):
                            wt = wp.tile([128, WIN], F32, tag="wt")
                            nc.gpsimd.dma_start(
                                wt[:], h2t_d[h, :, bass.ds(sv, WIN)])
                            nc.vector.reduce_max(
                                gslots[:, h * GR * WPG + w:
                                       h * GR * WPG + w + 1],
                                wt[:], axis=mybir.AxisListType.X)
        for h in range(2):
            for j in range(GR):
                nc.vector.reduce_max(
                    gm[:, h * GR + j:h * GR + j + 1],
                    gslots[:, h * GR * WPG + j * WPG:
                           h * GR * WPG + (j + 1) * WPG],
                    axis=mybir.AxisListType.X)
        nc.sync.dma_start(
            gmax_d[:, :, :].transpose([1, 0, 2]),
            gm[:].rearrange("p (h j) -> p h j", h=2))
        tc.strict_bb_all_engine_barrier()
        nc.gpsimd.collective_compute(
            "AllGather", mybir.AluOpType.bypass, replica_groups=RG,
            ins=[gmax_d[:, :, :]], outs=[gpool_d[:, :, :, :]])
        tc.strict_bb_all_engine_barrier()

        if debug_outs:
            nc.sync.dma_start(dbg["dbg_gmax"][:, :, :], gmax_d[:, :, :])
            nc.sync.dma_start(dbg["dbg_gpool"][:, :, :, :], gpool_d[:, :, :, :])
        # ---- FC head (replicated)
        gts = []
        for h in range(2):
            gt = cp.tile([128, G], F32, tag=f"gt{h}")
            nc.sync.dma_start(
                gt[:].rearrange("p (r j) -> p r j", r=R),
                gpool_d[:, h, :, :].transpose([1, 0, 2]))
            gts.append(gt)
        o1 = []
        for m in range(4):
            pfc = mmp.tile([128, G], F32, tag="p1")
            for h in range(2):
                nc.tensor.matmul(
                    pfc[:], f1ws[:, h * FCD + m * 128: h * FCD + (m + 1) * 128],
                    gts[h][:], start=(h == 0), stop=(h == 1))
            o1m = cp.tile([128, G], F32, tag=f"o1_{m}")
            nc.vector.tensor_scalar_add(o1m[:], pfc[:], f1bs[:, m:m + 1])
            o1.append(o1m)
        pfc2 = mmp.tile([G, PROJ], F32, tag="p1")
        for m in range(4):
            nc.tensor.matmul(pfc2[:], o1[m][:], f2ws[:, m * PROJ:(m + 1) * PROJ],
                             start=(m == 0), stop=False)
        nc.tensor.matmul(pfc2[:], ones[:, :G], f2bs[:], start=False, stop=True)
        osb = cp.tile([G, PROJ], F32)
        nc.vector.tensor_copy(osb[:], pfc2[:])
        nc.sync.dma_start(out_d[:, :], osb[:])

    nc.compile()
    return nc


# -------------------------------------------------------------------- runner

def _timed_spmd(nc, in_maps, iters=3):
    """Mirror bass2jax.run_bass_via_pjrt's multi-core path, but pre-place
    inputs on device and time warm executions (no NTFF hook in this
    container, so wall-clock around the PJRT execute is the HW metric)."""
    import time as _time

    import jax
    from jax.sharding import Mesh, NamedSharding, PartitionSpec
    from jax.experimental.shard_map import shard_map

    from concourse import bass2jax as b2j

    b2j.install_neuronx_cc_hook()
    n_cores = len(in_maps)
    partition_name = (nc.partition_id_tensor.name
                      if nc.partition_id_tensor else None)
    in_names, out_names, out_avals, zero_outs = [], [], [], []
    for alloc in nc.m.functions[0].allocations:
        if not isinstance(alloc, mybir.MemoryLocationSet):
            continue
        name = alloc.memorylocations[0].name
        if alloc.kind == "ExternalInput":
            if name != partition_name:
                in_names.append(name)
        elif alloc.kind == "ExternalOutput":
            shape = tuple(alloc.tensor_shape)
            dtype = mybir.dt.np(alloc.dtype)
            out_names.append(name)
            out_avals.append(jax.core.ShapedArray(shape, dtype))
            zero_outs.append(np.zeros(shape, dtype))
    n_params = len(in_names)
    n_outs = len(out_avals)
    all_in = list(in_names) + list(out_names)
    if partition_name is not None:
        all_in.append(partition_name)
    donate = tuple(range(n_params, n_params + n_outs))

    def _body(*args):
        operands = list(args)
        if partition_name is not None:
            operands.append(b2j.partition_id_tensor())
        outs = b2j._bass_exec_p.bind(
            *operands,
            out_avals=tuple(out_avals),
            in_names=tuple(all_in),
            out_names=tuple(out_names),
            lowering_input_output_aliases=(),
            sim_require_finite=True,
            sim_require_nnan=True,
            nc=nc,
        )
        return tuple(outs)

    devices = jax.devices()[:n_cores]
    mesh = Mesh(np.asarray(devices), ("core",))
    spec = NamedSharding(mesh, PartitionSpec("core"))
    in_specs = (PartitionSpec("core"),) * (n_params + n_outs)
    out_specs = (PartitionSpec("core"),) * n_outs
    sharded = jax.jit(
        shard_map(_body, mesh=mesh, in_specs=in_specs, out_specs=out_specs,
                  check_rep=False),
        donate_argnums=donate, keep_unused=True)

    concat_in = [
        np.concatenate([np.asarray(in_maps[c][k]) for c in range(n_cores)],
                       axis=0)
        for k in in_names
    ]
    tot_mb = sum(a.nbytes for a in concat_in) / 1e6
    print(f"[timed] transferring {tot_mb:.0f} MB inputs", flush=True)
    in_dev = [jax.device_put(a, spec) for a in concat_in]
    jax.block_until_ready(in_dev)
    print("[timed] inputs on device", flush=True)

    times = []
    out_arrs = None
    for _ in range(iters):
        zdev = [jax.device_put(
            np.zeros((n_cores * z.shape[0], *z.shape[1:]), z.dtype), spec)
            for z in zero_outs]
        jax.block_until_ready(zdev)
        t0 = _time.perf_counter()
        out_arrs = sharded(*in_dev, *zdev)
        jax.block_until_ready(out_arrs)
        times.append(_time.perf_counter() - t0)
        print(f"[timed] iter done {times[-1]*1e3:.2f} ms", flush=True)
    results = [
        {name: np.asarray(out_arrs[i]).reshape(n_cores, *out_avals[i].shape)[c]
         for i, name in enumerate(out_names)}
        for c in range(n_cores)
    ]
    return results, times


def kernel(x, edge_index, batch, W1, b1, W2, b2, fc1_w, fc1_b, fc2_w, fc2_b,
           te=2048, _timing=False, _iters=4, _variant=""):
    from concourse.bass_utils import run_bass_kernel_spmd

    cfg, per_core = preprocess(x, edge_index, batch, te=te)
    in_maps = make_in_maps(cfg, per_core, x, W1, b1, W2, b2,
                           fc1_w, fc1_b, fc2_w, fc2_b)
    nc = build_program(cfg, variant=_variant)
    if _timing:
        results, times = _timed_spmd(nc, in_maps, iters=_iters)
        return np.asarray(results[0]["out"], dtype=np.float32), times
    res = run_bass_kernel_spmd(nc, in_maps, list(range(R)))
    return np.asarray(res.results[0]["out"], dtype=np.float32)



# revision 13
# speedup vs baseline: 1.3483x; 1.3483x over previous
"""GCN encoder (2x GCNConv + global max pool + 2x FC) on 8 TRN2 NeuronCores.

Strategy (node partitioning, graph-aligned so pooling is core-local):
  - Nodes sharded contiguously at graph boundaries: rank r owns graphs
    [r*GR, (r+1)*GR) and a contiguous node range, padded to NMAX (mult 128).
  - Edges are routed to the rank owning their dst node.
  - Aggregation = gather + one-hot matmul (NO dma_scatter_add): edges are
    sorted by (gather-chunk, dst-slab, src-token) and padded so every
    128-edge group is (chunk, slab)-pure with a group count uniform across
    ranks (static SPMD program; per-rank variation lives only in the
    es/dstrel input data). Per group: SWDGE dma_gather fetches the
    (pre-scaled) feature rows edge-major; DVE builds a one-hot [128e,128d]
    via is_equal against an iota row; PE matmul accumulates
    onehot^T @ msg into a PSUM tile per (chunk,slab) run; DVE adds the
    finished run into the SBUF agg slab. This moves the scatter half of
    message passing off the Q7 descriptor-generation path (the previous
    bottleneck: ~6ns/edge serialized on GpSimd for EACH of gather+scatter).
  - Gather index tokens are int16, so each layer's table is split into 4
    chunks (token = row - chunk_base < 32766); a gather window (2048 slots)
    never crosses a chunk boundary.
  - GCN normalization: norm = dinv[src]*dinv[dst]. Tables are pre-scaled by
    dinv (x host-side; h1s on device), the dinv[dst] factor is applied
    per-node in the post phase:  out_l = relu((dinv.*(agg + dinv.*x)) @ W + b)
  - After layer 1, h1s = dinv .* h1 is AllGather'd (halo is everything for
    a random graph); layer 2 gathers from the [R*NMAX, F] shared table.
  - Max pooling: h2 is produced feature-major (h2T); per-graph max via
    clamped fixed-width windows (idempotent overlap), window starts are
    per-core data loaded into registers. Tiny AllGather of pooled partials,
    then the replicated FC head runs on every core.
"""

import math
from contextlib import ExitStack

import numpy as np

import concourse.bass as bass
import concourse.bacc as bacc
import concourse.mybir as mybir
import concourse.tile as tile
from concourse import library_config

F32 = mybir.dt.float32
I16 = mybir.dt.int16

R = 8          # NeuronCores
CH = 4         # gather chunks per table (int16 token range)
F = 128        # in dim == hidden
H2 = 256       # 2*hidden
FCD = 512      # fc1 out
PROJ = 128     # fc2 out
# Gather window (slots per dma_gather). Each gathered element consumes an
# m2s+s2m descriptor PAIR in the SWDGE ring (dynamic_dma_scratch_size/16
# entries); two windows are in flight, so 2*2*TW must stay under the ring.
TW = 1024
TW16 = TW // 16
TW128 = TW // 128


# ----------------------------------------------------------------- host prep

def _wrap16(a: np.ndarray) -> np.ndarray:
    """[T] int -> [128, T//16] int16: idx j at (j%16, j//16), replicated to
    all 8 groups of 16 partitions."""
    T = a.shape[0]
    w = np.ascontiguousarray(a.reshape(T // 16, 16).T).astype(np.int16)
    return np.tile(w, (8, 1))


def preprocess(x, edge_index, batch, win=None):
    x = np.asarray(x, dtype=np.float32)
    src = np.asarray(edge_index[0], dtype=np.int64)
    dst = np.asarray(edge_index[1], dtype=np.int64)
    batch = np.asarray(batch, dtype=np.int64)
    N = x.shape[0]
    G_ = int(batch.max()) + 1 if batch.size else 1
    G_ = max(G_, R)
    assert G_ % R == 0, f"graphs {G_} not divisible by {R}"
    GR = G_ // R

    sizes = np.bincount(batch, minlength=G_)
    gstart = np.concatenate([[0], np.cumsum(sizes)])
    rb = gstart[::GR].copy()                          # [R+1] rank node bounds
    assert rb[-1] == N
    Nc = np.diff(rb)
    NMAX = int(math.ceil(max(int(Nc.max()), 128) / 128) * 128)
    NTL = NMAX // 128

    deg = np.bincount(dst, minlength=N).astype(np.float64) + 1.0
    dinv = (1.0 / np.sqrt(deg)).astype(np.float32)

    rk = np.searchsorted(rb, dst, side="right") - 1
    dstrel = dst - rb[rk]
    slab = dstrel // 128
    d128 = dstrel % 128

    # layer tokens
    tok1 = src
    rks = np.searchsorted(rb, src, side="right") - 1
    tok2 = rks * NMAX + (src - rb[rks])

    def build(tok, N_table):
        CHS = int(math.ceil(N_table / CH))
        assert CHS <= 32766, CHS
        ch = tok // CHS
        key = (rk * CH + ch) * NTL + slab
        bc = np.bincount(key, minlength=R * CH * NTL)
        counts = bc.reshape(R, CH, NTL)
        Gg = np.ceil(counts.max(axis=0) / 128).astype(np.int64)

        runs = []
        win_chunk = []
        NG = 0
        grp_start = np.zeros((CH, NTL), dtype=np.int64)
        for c in range(CH):
            sec = 0
            for s in range(NTL):
                g = int(Gg[c, s])
                if g == 0:
                    continue
                grp_start[c, s] = NG
                runs.append((c, s, g))
                NG += g
                sec += g
            padg = (-sec) % TW128
            if padg:
                runs.append((c, NTL - 1, padg))
                NG += padg
                sec += padg
            win_chunk += [c] * (sec // TW128)
        NW = len(win_chunk)

        tokens = np.zeros((R, NG * 128), dtype=np.int64)
        drel = np.full((R, NG * 128), -1.0, dtype=np.float32)
        rel = tok - ch * CHS
        order = np.lexsort((rel, slab.astype(np.int64), ch, rk))
        rk_s = rk[order]; ch_s = ch[order]; sl_s = slab[order]
        rel_s = rel[order]; d_s = d128[order]
        # position within (r, c, s) run
        key_s = (rk_s * CH + ch_s) * NTL + sl_s
        uniq, first_idx = np.unique(key_s, return_index=True)
        pos = np.arange(len(key_s)) - np.repeat(
            first_idx, np.diff(np.r_[first_idx, len(key_s)]))
        slot = grp_start[ch_s, sl_s] * 128 + pos
        tokens[rk_s, slot] = rel_s
        drel[rk_s, slot] = d_s.astype(np.float32)

        es = np.zeros((R, NW, 128, TW16), dtype=np.int16)
        dr = np.zeros((R, 128, NG), dtype=np.float32)
        for r in range(R):
            tw = tokens[r].reshape(NW, TW)
            for w in range(NW):
                es[r, w] = _wrap16(tw[w])
            dr[r] = drel[r].reshape(NG, 128).T
        cb = [min(c * CHS, N_table) for c in range(CH + 1)]
        return dict(CHS=CHS, runs=runs, NW=NW, win_chunk=win_chunk, NG=NG,
                    cbase=cb), es, dr

    m1, es1, dr1 = build(tok1, N)
    m2, es2, dr2 = build(tok2, R * NMAX)

    # pre-scaled gather table: xg = dinv .* x
    xg = (dinv[:, None] * x).astype(np.float32)

    # per-rank padded node slices for the post phase
    xc = np.zeros((R, NMAX, F), dtype=np.float32)
    dvc = np.zeros((R, NMAX, 1), dtype=np.float32)
    for r in range(R):
        n = int(Nc[r])
        xc[r, :n] = xg[rb[r]:rb[r + 1]]
        dvc[r, :n, 0] = dinv[rb[r]:rb[r + 1]]

    # pooling windows (unchanged from baseline)
    min_sz = int(sizes.min())
    assert min_sz > 0, "empty graph not supported"
    if win is None:
        win = 512
    win = int(min(win, min_sz))
    wpg = int(math.ceil(int(sizes.max()) / win))
    wstart = np.zeros((R, GR * wpg), dtype=np.int32)
    for r in range(R):
        for j in range(GR):
            g = r * GR + j
            a0 = int(gstart[g] - rb[r])
            sz = int(sizes[g])
            for k in range(wpg):
                wstart[r, j * wpg + k] = min(a0 + k * win, a0 + sz - win)

    cfg = dict(N=N, G=G_, GR=GR, NMAX=NMAX, NTL=NTL, WIN=win, WPG=wpg,
               m1=m1, m2=m2)
    per_core = dict(es1=es1, dr1=dr1, es2=es2, dr2=dr2,
                    xc=xc, dvc=dvc, xg=xg,
                    wstart=wstart[:, :, None].astype(np.int32))
    return cfg, per_core


def make_in_maps(cfg, per_core, x, W1, b1, W2, b2, fc1_w, fc1_b, fc2_w, fc2_b):
    w1 = np.asarray(W1, dtype=np.float32)
    b1v = np.asarray(b1, dtype=np.float32).reshape(1, F)
    w2 = np.asarray(W2, dtype=np.float32)
    b2s = np.asarray(b2, dtype=np.float32).reshape(2, 128).T.copy()      # [128,2]
    f1w = np.asarray(fc1_w, dtype=np.float32).reshape(2, 128, FCD)
    f1w = np.ascontiguousarray(f1w.transpose(1, 0, 2)).reshape(128, 2 * FCD)
    f1b = np.asarray(fc1_b, dtype=np.float32).reshape(4, 128).T.copy()   # [128,4]
    f2w = np.asarray(fc2_w, dtype=np.float32).reshape(4, 128, PROJ)
    f2w = np.ascontiguousarray(f2w.transpose(1, 0, 2)).reshape(128, 4 * PROJ)
    f2b = np.asarray(fc2_b, dtype=np.float32).reshape(1, PROJ)
    ident = np.eye(128, dtype=np.float32)
    iota = np.tile(np.arange(128, dtype=np.float32)[None, :], (128, 1))

    shared = dict(x=per_core["xg"], w1=w1, b1=b1v, w2=w2, b2s=b2s,
                  f1w=f1w, f1b=f1b, f2w=f2w, f2b=f2b, ident=ident, iota=iota)
    in_maps = []
    for r in range(R):
        m = dict(shared)
        m["xc"] = per_core["xc"][r]
        m["dvc"] = per_core["dvc"][r]
        m["es1"] = per_core["es1"][r]
        m["dr1"] = per_core["dr1"][r]
        m["es2"] = per_core["es2"][r]
        m["dr2"] = per_core["dr2"][r]
        m["wstart"] = per_core["wstart"][r]
        in_maps.append(m)
    return in_maps


# ------------------------------------------------------------------- builder

def build_program(cfg, variant=""):
    N = cfg["N"]; G = cfg["G"]; GR = cfg["GR"]
    NMAX = cfg["NMAX"]; NTL = cfg["NTL"]
    WIN = cfg["WIN"]; WPG = cfg["WPG"]
    m1 = cfg["m1"]; m2 = cfg["m2"]
    RG = [list(range(R))]

    nc = bacc.Bacc("TRN2", target_bir_lowering=False, debug=False,
                   num_devices=R, dynamic_dma_scratch_size=81920)

    x_d = nc.dram_tensor("x", [N, F], F32, kind="ExternalInput")
    xc_d = nc.dram_tensor("xc", [NMAX, F], F32, kind="ExternalInput")
    dvc_d = nc.dram_tensor("dvc", [NMAX, 1], F32, kind="ExternalInput")
    es1_d = nc.dram_tensor("es1", [m1["NW"], 128, TW16], I16,
                           kind="ExternalInput")
    dr1_d = nc.dram_tensor("dr1", [128, m1["NG"]], F32, kind="ExternalInput")
    es2_d = nc.dram_tensor("es2", [m2["NW"], 128, TW16], I16,
                           kind="ExternalInput")
    dr2_d = nc.dram_tensor("dr2", [128, m2["NG"]], F32, kind="ExternalInput")
    w1_d = nc.dram_tensor("w1", [F, F], F32, kind="ExternalInput")
    b1_d = nc.dram_tensor("b1", [1, F], F32, kind="ExternalInput")
    w2_d = nc.dram_tensor("w2", [F, H2], F32, kind="ExternalInput")
    b2s_d = nc.dram_tensor("b2s", [128, 2], F32, kind="ExternalInput")
    f1w_d = nc.dram_tensor("f1w", [128, 2 * FCD], F32, kind="ExternalInput")
    f1b_d = nc.dram_tensor("f1b", [128, 4], F32, kind="ExternalInput")
    f2w_d = nc.dram_tensor("f2w", [128, 4 * PROJ], F32, kind="ExternalInput")
    f2b_d = nc.dram_tensor("f2b", [1, PROJ], F32, kind="ExternalInput")
    ident_d = nc.dram_tensor("ident", [128, 128], F32, kind="ExternalInput")
    iota_d = nc.dram_tensor("iota", [128, 128], F32, kind="ExternalInput")
    wstart_d = nc.dram_tensor("wstart", [GR * WPG, 1], mybir.dt.int32,
                              kind="ExternalInput")

    h1s_d = nc.dram_tensor("h1s", [NMAX, F], F32)
    h1full_d = nc.dram_tensor("h1full", [R * NMAX, F], F32,
                              addr_space="Shared")
    h2t_d = nc.dram_tensor("h2t", [2, 128, NMAX], F32)
    gmax_d = nc.dram_tensor("gmax", [2, 128, GR], F32)
    gpool_d = nc.dram_tensor("gpool", [R, 2, 128, GR], F32, addr_space="Shared")
    out_d = nc.dram_tensor("out", [G, PROJ], F32, kind="ExternalOutput")

    with tile.TileContext(nc, num_cores=R) as tc, ExitStack() as stk:
        cp = stk.enter_context(tc.tile_pool(name="consts", bufs=1))
        w1s = cp.tile([F, F], F32)
        b1s = cp.tile([1, F], F32)
        w2s = cp.tile([F, H2], F32)
        b2ss = cp.tile([128, 2], F32)
        f1ws = cp.tile([128, 2 * FCD], F32)
        f1bs = cp.tile([128, 4], F32)
        f2ws = cp.tile([128, 4 * PROJ], F32)
        f2bs = cp.tile([1, PROJ], F32)
        idents = cp.tile([128, 128], F32)
        iotas = cp.tile([128, 128], F32)
        ones = cp.tile([1, 128], F32)
        agg = cp.tile([128, NTL * F], F32, name="agg")  # node-major agg slabs

        nc.gpsimd.load_library(library_config.mlp)
        nc.sync.dma_start(w1s[:], w1_d[:])
        nc.sync.dma_start(b1s[:], b1_d[:])
        nc.sync.dma_start(w2s[:], w2_d[:])
        nc.sync.dma_start(b2ss[:], b2s_d[:])
        nc.sync.dma_start(f1ws[:], f1w_d[:])
        nc.sync.dma_start(f1bs[:], f1b_d[:])
        nc.sync.dma_start(f2ws[:], f2w_d[:])
        nc.sync.dma_start(f2bs[:], f2b_d[:])
        nc.sync.dma_start(idents[:], ident_d[:])
        nc.sync.dma_start(iotas[:], iota_d[:])
        nc.vector.memset(ones[:], 1.0)

        ip = stk.enter_context(tc.tile_pool(name="idx", bufs=3))
        mp = stk.enter_context(tc.tile_pool(name="msg", bufs=3))
        ohp = stk.enter_context(tc.tile_pool(name="oh", bufs=3))
        agp = stk.enter_context(tc.tile_pool(name="agps", bufs=2,
                                             space="PSUM"))
        drt = [cp.tile([128, m["NG"]], F32, name=f"drt{i}")
               for i, m in ((0, m1), (1, m2))]
        nc.sync.dma_start(drt[0][:], dr1_d[:])
        nc.sync.dma_start(drt[1][:], dr2_d[:])

        def edge_pass(table_d, meta, es_d, dr_s, lbl=""):
            """Gather + one-hot matmul aggregation into `agg` (zeroed).
            Gathers are emitted with 2-window prefetch ahead of the group
            consumers so the Q7 descriptor-gen stream never waits."""
            cb = meta["cbase"]
            NW = meta["NW"]
            msgs = [None] * NW
            state = {"next_w": 0}

            def emit_gather():
                w = state["next_w"]
                c = meta["win_chunk"][w]
                sidx = ip.tile([128, TW16], I16, tag="es")
                nc.sync.dma_start(sidx[:], es_d[w])
                msg = mp.tile([128, TW128, 128], F32, tag="msg")
                if lbl == "" and w < 3:  # init pool slots once
                    nc.vector.memset(msg[:], 0.0)
                tab = table_d[cb[c]:cb[c + 1], :]
                nc.gpsimd.dma_gather(msg[:], tab, sidx[:], TW, TW, F)
                msgs[w] = msg
                state["next_w"] += 1

            # group schedule: walk runs, matmul-accumulate per run, flush
            g0 = 0
            for (c, s, ng) in meta["runs"]:
                ps = agp.tile([128, 128], F32, tag="agps")
                for k in range(ng):
                    g = g0 + k
                    w, p = g // TW128, g % TW128
                    while state["next_w"] <= min(w + 1, NW - 1):
                        emit_gather()
                    oh = ohp.tile([128, 128], F32, tag="oh")
                    nc.vector.tensor_scalar(
                        oh[:], iotas[:], dr_s[:, g:g + 1], None,
                        mybir.AluOpType.is_equal)
                    nc.tensor.matmul(ps[:], oh[:], msgs[w][:, p, :],
                                     start=(k == 0), stop=(k == ng - 1))
                sl = agg[:, s * F:(s + 1) * F]
                nc.vector.tensor_add(sl, sl, ps[:])
                g0 += ng

        # ---- layer 1
        nc.vector.memset(agg[:], 0.0)
        pp = stk.enter_context(tc.tile_pool(name="post", bufs=3))
        tp = stk.enter_context(
            tc.tile_pool(name="tpsum", bufs=2, space="PSUM"))
        mmp = stk.enter_context(
            tc.tile_pool(name="mpsum", bufs=2, space="PSUM"))

        with nc.named_scope("edge1"):
            edge_pass(x_d, m1, es1_d, drt[0])
            tc.strict_bb_all_engine_barrier()

        # ---- post 1: h1s = dinv*relu((dinv*(agg + dinv*x)) @ W1 + b1)
        with nc.named_scope("post1"):
            for i in range(NTL):
                sl = slice(i * 128, (i + 1) * 128)
                xt = pp.tile([128, F], F32, tag="xt")
                nc.sync.dma_start(xt[:], xc_d[sl, :])
                dv = pp.tile([128, 1], F32, tag="dv")
                nc.sync.dma_start(dv[:], dvc_d[sl, :])
                s = pp.tile([128, F], F32, tag="s")
                nc.vector.tensor_add(s[:], xt[:], agg[:, i * F:(i + 1) * F])
                s2 = pp.tile([128, F], F32, tag="s2")
                nc.vector.tensor_scalar_mul(s2[:], s[:], dv[:])
                pt = tp.tile([128, 128], F32, tag="pt")
                nc.tensor.transpose(pt[:], s2[:], idents[:])
                aT = pp.tile([128, 128], F32, tag="aT")
                nc.vector.tensor_copy(aT[:], pt[:])
                p1 = mmp.tile([128, F], F32, tag="p1")
                nc.tensor.matmul(p1[:], aT[:], w1s[:], start=True, stop=False)
                nc.tensor.matmul(p1[:], ones[:], b1s[:], start=False,
                                 stop=True)
                h1 = pp.tile([128, F], F32, tag="h1")
                nc.scalar.activation(h1[:], p1[:],
                                     mybir.ActivationFunctionType.Relu)
                h1s = pp.tile([128, F], F32, tag="h1s")
                nc.vector.tensor_scalar_mul(h1s[:], h1[:], dv[:])
                nc.sync.dma_start(h1s_d[sl, :], h1s[:])
            tc.strict_bb_all_engine_barrier()

        with nc.named_scope("allgather"):
            nc.vector.memset(agg[:], 0.0)  # overlaps the collective
            nc.gpsimd.collective_compute(
                "AllGather", mybir.AluOpType.bypass, replica_groups=RG,
                ins=[h1s_d[:, :]], outs=[h1full_d[:, :]])
            tc.strict_bb_all_engine_barrier()

        # ---- layer 2
        with nc.named_scope("edge2"):
            edge_pass(h1full_d, m2, es2_d, drt[1], lbl="b")
            tc.strict_bb_all_engine_barrier()

        # ---- post 2: h2T = relu(W2^T @ (dinv*(agg2 + h1s)) + b2), feature-major
        _sc_p2, _ = nc.enter_named_scope("post2", False)
        for i in range(NTL):
            sl = slice(i * 128, (i + 1) * 128)
            hs = pp.tile([128, F], F32, tag="hsb")
            nc.sync.dma_start(hs[:], h1s_d[sl, :])
            dv = pp.tile([128, 1], F32, tag="dv")
            nc.sync.dma_start(dv[:], dvc_d[sl, :])
            s = pp.tile([128, F], F32, tag="s")
            nc.vector.tensor_add(s[:], hs[:], agg[:, i * F:(i + 1) * F])
            s2 = pp.tile([128, F], F32, tag="s2")
            nc.vector.tensor_scalar_mul(s2[:], s[:], dv[:])
            pt = tp.tile([128, 128], F32, tag="pt")
            nc.tensor.transpose(pt[:], s2[:], idents[:])
            aT = pp.tile([128, 128], F32, tag="aT")
            nc.vector.tensor_copy(aT[:], pt[:])
            for h in range(2):
                p2 = mmp.tile([128, 128], F32, tag="p1")
                nc.tensor.matmul(p2[:], w2s[:, h * 128:(h + 1) * 128], aT[:],
                                 start=True, stop=True)
                h2t = pp.tile([128, 128], F32, tag="h1")
                nc.scalar.activation(h2t[:], p2[:],
                                     mybir.ActivationFunctionType.Relu,
                                     bias=b2ss[:, h:h + 1])
                nc.sync.dma_start(h2t_d[h, :, sl], h2t[:])

        tc.strict_bb_all_engine_barrier()
        nc.leave_named_scope("post2", _sc_p2, False)

        # ---- pooling: WPG fixed windows per graph, data-driven start columns
        _sc_pool, _ = nc.enter_named_scope("pool", False)
        gm = cp.tile([128, 2 * GR], F32)
        wp = stk.enter_context(tc.tile_pool(name="win", bufs=4))
        wsts = cp.tile([GR * WPG, 1], mybir.dt.int32)
        nc.sync.dma_start(wsts[:], wstart_d[:])
        gslots = cp.tile([128, 2 * GR * WPG], F32)
        for j in range(GR):
            for k in range(WPG):
                w = j * WPG + k
                with nc.gpsimd.register(f"wst_{w}") as rg:
                    nc.gpsimd.reg_load(rg, wsts[w:w + 1, 0:1])
                    sv = nc.gpsimd.snap(rg)
                    for h in range(2):
                        wt = wp.tile([128, WIN], F32, tag="wt")
                        nc.gpsimd.dma_start(
                            wt[:], h2t_d[h, :, bass.ds(sv, WIN)])
                        nc.vector.reduce_max(
                            gslots[:, h * GR * WPG + w:
                                   h * GR * WPG + w + 1],
                            wt[:], axis=mybir.AxisListType.X)
        for h in range(2):
            for j in range(GR):
                nc.vector.reduce_max(
                    gm[:, h * GR + j:h * GR + j + 1],
                    gslots[:, h * GR * WPG + j * WPG:
                           h * GR * WPG + (j + 1) * WPG],
                    axis=mybir.AxisListType.X)
        nc.sync.dma_start(
            gmax_d[:, :, :].transpose([1, 0, 2]),
            gm[:].rearrange("p (h j) -> p h j", h=2))
        tc.strict_bb_all_engine_barrier()
        nc.gpsimd.collective_compute(
            "AllGather", mybir.AluOpType.bypass, replica_groups=RG,
            ins=[gmax_d[:, :, :]], outs=[gpool_d[:, :, :, :]])
        tc.strict_bb_all_engine_barrier()
        nc.leave_named_scope("pool", _sc_pool, False)

        # ---- FC head (replicated)
        _sc_fc, _ = nc.enter_named_scope("fc", False)
        gts = []
        for h in range(2):
            gt = cp.tile([128, G], F32, tag=f"gt{h}")
            nc.sync.dma_start(
                gt[:].rearrange("p (r j) -> p r j", r=R),
                gpool_d[:, h, :, :].transpose([1, 0, 2]))
            gts.append(gt)
        o1 = []
        for m in range(4):
            pfc = mmp.tile([128, G], F32, tag="p1")
            for h in range(2):
                nc.tensor.matmul(
                    pfc[:], f1ws[:, h * FCD + m * 128: h * FCD + (m + 1) * 128],
                    gts[h][:], start=(h == 0), stop=(h == 1))
            o1m = cp.tile([128, G], F32, tag=f"o1_{m}")
            nc.vector.tensor_scalar_add(o1m[:], pfc[:], f1bs[:, m:m + 1])
            o1.append(o1m)
        pfc2 = mmp.tile([G, PROJ], F32, tag="p1")
        for m in range(4):
            nc.tensor.matmul(pfc2[:], o1[m][:], f2ws[:, m * PROJ:(m + 1) * PROJ],
                             start=(m == 0), stop=False)
        nc.tensor.matmul(pfc2[:], ones[:, :G], f2bs[:], start=False, stop=True)
        osb = cp.tile([G, PROJ], F32)
        nc.vector.tensor_copy(osb[:], pfc2[:])
        nc.sync.dma_start(out_d[:, :], osb[:])
        nc.leave_named_scope("fc", _sc_fc, False)

    nc.compile()
    return nc


# -------------------------------------------------------------------- runner

def _timed_spmd(nc, in_maps, iters=3):
    """Pre-place inputs on device and time warm executions (wall-clock
    around the PJRT execute is the HW metric)."""
    import time as _time

    import jax
    from jax.sharding import Mesh, NamedSharding, PartitionSpec
    from jax.experimental.shard_map import shard_map

    from concourse import bass2jax as b2j

    b2j.install_neuronx_cc_hook()
    n_cores = len(in_maps)
    partition_name = (nc.partition_id_tensor.name
                      if nc.partition_id_tensor else None)
    in_names, out_names, out_avals, zero_outs = [], [], [], []
    for alloc in nc.m.functions[0].allocations:
        if not isinstance(alloc, mybir.MemoryLocationSet):
            continue
        name = alloc.memorylocations[0].name
        if alloc.kind == "ExternalInput":
            if name != partition_name:
                in_names.append(name)
        elif alloc.kind == "ExternalOutput":
            shape = tuple(alloc.tensor_shape)
            dtype = mybir.dt.np(alloc.dtype)
            out_names.append(name)
            out_avals.append(jax.core.ShapedArray(shape, dtype))
            zero_outs.append(np.zeros(shape, dtype))
    n_params = len(in_names)
    n_outs = len(out_avals)
    all_in = list(in_names) + list(out_names)
    if partition_name is not None:
        all_in.append(partition_name)
    donate = tuple(range(n_params, n_params + n_outs))

    def _body(*args):
        operands = list(args)
        if partition_name is not None:
            operands.append(b2j.partition_id_tensor())
        outs = b2j._bass_exec_p.bind(
            *operands,
            out_avals=tuple(out_avals),
            in_names=tuple(all_in),
            out_names=tuple(out_names),
            lowering_input_output_aliases=(),
            sim_require_finite=True,
            sim_require_nnan=True,
            nc=nc,
        )
        return tuple(outs)

    devices = jax.devices()[:n_cores]
    mesh = Mesh(np.asarray(devices), ("core",))
    spec = NamedSharding(mesh, PartitionSpec("core"))
    in_specs = (PartitionSpec("core"),) * (n_params + n_outs)
    out_specs = (PartitionSpec("core"),) * n_outs
    sharded = jax.jit(
        shard_map(_body, mesh=mesh, in_specs=in_specs, out_specs=out_specs,
                  check_rep=False),
        donate_argnums=donate, keep_unused=True)

    concat_in = [
        np.concatenate([np.asarray(in_maps[c][k]) for c in range(n_cores)],
                       axis=0)
        for k in in_names
    ]
    tot_mb = sum(a.nbytes for a in concat_in) / 1e6
    print(f"[timed] transferring {tot_mb:.0f} MB inputs", flush=True)
    in_dev = [jax.device_put(a, spec) for a in concat_in]
    jax.block_until_ready(in_dev)
    print("[timed] inputs on device", flush=True)

    times = []
    out_arrs = None
    for _ in range(iters):
        zdev = [jax.device_put(
            np.zeros((n_cores * z.shape[0], *z.shape[1:]), z.dtype), spec)
            for z in zero_outs]
        jax.block_until_ready(zdev)
        t0 = _time.perf_counter()
        out_arrs = sharded(*in_dev, *zdev)
        jax.block_until_ready(out_arrs)
        times.append(_time.perf_counter() - t0)
        print(f"[timed] iter done {times[-1]*1e3:.2f} ms", flush=True)
    results = [
        {name: np.asarray(out_arrs[i]).reshape(n_cores, *out_avals[i].shape)[c]
         for i, name in enumerate(out_names)}
        for c in range(n_cores)
    ]
    return results, times


def kernel(x, edge_index, batch, W1, b1, W2, b2, fc1_w, fc1_b, fc2_w, fc2_b,
           _timing=False, _iters=4, _variant=""):
    from concourse.bass_utils import run_bass_kernel_spmd

    cfg, per_core = preprocess(x, edge_index, batch)
    in_maps = make_in_maps(cfg, per_core, x, W1, b1, W2, b2,
                           fc1_w, fc1_b, fc2_w, fc2_b)
    nc = build_program(cfg, variant=_variant)
    if _timing:
        results, times = _timed_spmd(nc, in_maps, iters=_iters)
        return np.asarray(results[0]["out"], dtype=np.float32), times
    res = run_bass_kernel_spmd(nc, in_maps, list(range(R)))
    return np.asarray(res.results[0]["out"], dtype=np.float32)
